# revision 11
# baseline (speedup 1.0000x reference)
"""MoE expert-collection kernel for 8 Trainium2 NeuronCores.

Problem (hardcoded shapes):
  x          [8192, 1024] f32
  expert_idx [8192]       int    (values 0..7)
  Wr         [8, 1024, 1024] f32, br [8, 1024] f32   (routing experts)
  Ws         [2, 1024, 1024] f32, bs [2, 1024] f32   (shared experts)
  out[n] = silu(x[n] @ Wr[e_n] + br[e_n]) + sum_s silu(x[n] @ Ws[s] + bs[s])

Strategy (expert parallel, host-side all-to-all):
  - Host sorts tokens by expert; core e computes silu(x @ Wr[e] + br[e]) for
    the tokens routed to expert e. Core e's xu = [sorted window e*S:(e+1)*S |
    extras], so one x load serves both the routed phase (all U columns with
    Wr[e]; only expert-e ones kept) and the shared phase (window cols 0:S).
  - Shared experts are data-parallel: core e computes
    silu(x @ Ws[0] + bs[0]) + silu(x @ Ws[1] + bs[1]) on the window.
  - Host combines: out = concat(shared slices); out[routed order] += routed.
  - Matmuls run in bf16 (fp32 PSUM accumulation).

Device schedule (measured-roofline driven):
  - Inputs ride two HWDGE queues concurrently (W0+W1 on the scalar queue,
    bias+xu+W2 on sync): ~350 GB/s aggregate vs ~300 single-queue, and the
    ramp's per-k needs (W0 k-tile + xu k-row) land pairwise every ~1.2us.
  - Ramp is k-outer over xu cols 0:512 (two 256-col chunks, 2 m-outputs
    packed per PSUM bank = all 8 banks): matmuls issue as each k-pair lands,
    so useful PE work starts ~1.4us after the first DMA instead of waiting
    ~12us for all of W0+xu.  N=256 with alternating lhsT streams at the
    109ns ideal (LDWEIGHTS fully hidden) per a microbenchmark probe.
  - One PSUM tag with bufs=8 and uniform [P,512] tiles: the ramp's 8 live
    accumulators hand banks to the steady/shared allocations in exactly the
    silu drain order.
  - Steady routed (cols 512:U) and the shared phase keep the m-outer
    N=512 streaming structure (measured at the 215ns/matmul roofline).
  - Routed silus write one [P, 8, U] f32 staging tile; a single 4.3MB store
    replaces 24 chunked stores.  outs is stored bf16 (tail store halves;
    ~0.1% extra relative error against a 2e-2 budget).
  - ~12 N=256 warmup matmuls on a memset tile pull the HAM clock un-throttle
    to ~11us so only the first couple of ramp k-steps run at 1.2 GHz.
"""

import contextlib
import ctypes
import math
import sys
import types

import numpy as np
import ml_dtypes

import concourse.mybir as mybir
import concourse.tile as tile
from concourse import bacc
from concourse import bass_utils

N_CORES = 8
D = 1024          # d_in == d_out
P = 128           # partitions
KT = D // P       # 8 k-tiles
NJ = 3            # matrices per core: Wr[e], Ws[0], Ws[1]
N_EXPERTS = 8
S = 8192 // N_CORES  # shared-slice tokens per core (1024)
RAMP = 512        # xu columns processed k-outer during the input stream

BF16 = mybir.dt.bfloat16
F32 = mybir.dt.float32

# exposed for test.py introspection
last_results = None
last_nc = None
last_in_maps = None

_program_cache = {}


def _install_ntff_hook_fallback():
    """Some containers (including this one) lack antenv.axon_hooks, but
    concourse's run_bass_kernel_spmd imports it unconditionally when tracing
    is requested (BASS_TRACE=1). Provide it: a ctypes port of
    trn_boot._ntff_profile_via_ctypes driving NRT profiling through the axon
    PJRT plugin, or a None hook (= trace gracefully skipped) if unavailable."""
    if "antenv.axon_hooks" in sys.modules:
        return
    try:
        import antenv.axon_hooks  # noqa: F401
        return
    except ImportError:
        pass

    hook = None
    try:
        lib = ctypes.CDLL("/opt/axon/libaxon_pjrt.so")
        if hasattr(lib, "axon_start_nrt_profile"):
            lib.axon_start_nrt_profile.argtypes = [
                ctypes.POINTER(ctypes.c_int64),
                ctypes.c_size_t,
            ]
            lib.axon_start_nrt_profile.restype = ctypes.c_int64
            lib.axon_stop_nrt_profile.argtypes = [ctypes.c_char_p]
            lib.axon_stop_nrt_profile.restype = ctypes.c_int64

            @contextlib.contextmanager
            def _hook(output_dir, device_ids):
                import jax

                jax.devices()  # force PJRT init so the axon client exists
                if device_ids:
                    ids = (ctypes.c_int64 * len(device_ids))(*device_ids)
                    rc = lib.axon_start_nrt_profile(ids, len(device_ids))
                else:
                    rc = lib.axon_start_nrt_profile(None, 0)
                if rc != 0:
                    raise RuntimeError(f"axon_start_nrt_profile rc={rc}")
                try:
                    yield
                finally:
                    n = lib.axon_stop_nrt_profile(str(output_dir).encode())
                    if n < 0:
                        raise RuntimeError(f"axon_stop_nrt_profile rc={n}")

            hook = _hook
    except OSError:
        pass

    mod = types.ModuleType("antenv.axon_hooks")
    mod.get_axon_ntff_profile_hook = lambda: hook
    mod.set_axon_ntff_profile_hook = lambda h: None
    sys.modules["antenv.axon_hooks"] = mod


_install_ntff_hook_fallback()


def _chunk_ranges(c0, C, chunk=512):
    out = []
    while c0 < C:
        c1 = min(c0 + chunk, C)
        out.append((c0, c1))
        c0 = c1
    return out


def _build_program(U):
    assert U >= RAMP + 1, f"U={U} too small for the ramp split"
    steady_chunks = _chunk_ranges(RAMP, U)   # [(512,1024),(1024,U)] normally
    s_chunks = _chunk_ranges(0, S)

    nc = bacc.Bacc(
        "TRN2",
        target_bir_lowering=False,
        debug=False,
        enable_asserts=False,
        num_devices=N_CORES,
    )
    xu_d = nc.dram_tensor("xu", [D, U], BF16, kind="ExternalInput")
    W_d = nc.dram_tensor("W", [NJ, D, D], BF16, kind="ExternalInput")
    b_d = nc.dram_tensor("b", [P, NJ * KT], F32, kind="ExternalInput")
    outr_d = nc.dram_tensor("outr", [D, U], F32, kind="ExternalOutput")
    outs_d = nc.dram_tensor("outs", [D, S], BF16, kind="ExternalOutput")

    with tile.TileContext(nc) as tc:
        with (
            tc.tile_pool(name="const", bufs=1) as constp,
            tc.tile_pool(name="wpool", bufs=1) as wp,
            tc.tile_pool(name="xpool", bufs=1) as xp,
            tc.tile_pool(name="silp", bufs=3) as silp,
            tc.tile_pool(name="outp", bufs=3) as outp,
            tc.tile_pool(name="psum", bufs=8, space="PSUM") as psump,
        ):
            def psum_tile(name):
                # uniform [P,512] f32 (= one bank) so the single-tag rotation
                # recycles the ramp's banks for the steady/shared phases
                return psump.tile([P, 512], F32, tag="ps", name=name)

            # warmup source: gpsimd is the first engine free (~6.9us), so it
            # memsets the tile slightly before the vector engine could
            warm_sb = constp.tile([P, 384], BF16, name="warm_sb")
            nc.gpsimd.memset(warm_sb[:], 0.0)
            bias_t = constp.tile([P, NJ * KT], F32)
            w_t = wp.tile([P, NJ, KT, D], BF16)
            xu_t = xp.tile([P, KT, U], BF16, name="xu_t")
            x_stage = outp.tile([P, KT, U], F32, tag="xstage", bufs=1,
                                name="x_stage")

            # --- input DMAs: two queues in parallel, first-use order ---
            # sync: xu rows (2112B lines, ramp-critical), W2
            # scalar: W0 k-tiles (ramp-critical), bias, W1
            # k=0 is split in halves so the ramp's first matmuls (m 0-3 on
            # cols 0:256) wait on ~0.2MB instead of ~0.53MB.
            nc.scalar.dma_start(w_t[:, 0, 0, :P], W_d[0, :P, :P])
            nc.sync.dma_start(xu_t[:, 0, :256], xu_d[:P, :256])
            nc.scalar.dma_start(w_t[:, 0, 0, P:512], W_d[0, :P, P:512])
            nc.sync.dma_start(xu_t[:, 0, 256:RAMP], xu_d[:P, 256:RAMP])
            nc.scalar.dma_start(w_t[:, 0, 0, 512:], W_d[0, :P, 512:])
            for k in range(1, KT):
                nc.scalar.dma_start(w_t[:, 0, k, :], W_d[0, k * P:(k + 1) * P, :])
                nc.sync.dma_start(xu_t[:, k, :RAMP], xu_d[k * P:(k + 1) * P, :RAMP])
            # steady-phase xu halves deferred behind the ramp-critical stream
            # (they are first needed ~8us after the last ramp k-pair)
            for k in range(KT):
                nc.sync.dma_start(
                    xu_t[:, k, RAMP:], xu_d[k * P:(k + 1) * P, RAMP:]
                )
            nc.scalar.dma_start(bias_t[:], b_d[:])
            nc.scalar.dma_start(
                w_t[:, 1, :, :], W_d[1].rearrange("(kt p) n -> p kt n", p=P)
            )
            nc.sync.dma_start(
                w_t[:, 2, :, :], W_d[2].rearrange("(kt p) n -> p kt n", p=P)
            )

            # --- PE warmup: keep the PE continuously busy from ~7.3us until
            # the first (now 96KB) k0 sub-unit lands (~9us) so the HAM
            # un-throttle fires early without an idle-gap reset ---
            warm_ps = psum_tile("warm_ps")
            for i in range(9):
                nc.tensor.matmul(
                    warm_ps[:, :256], warm_sb[:, :P], warm_sb[:, P:P + 256],
                    start=True, stop=True,
                )

            # --- routed ramp: k-outer over cols 0:512, two 256-col chunks,
            # 2 m-outputs per bank -> 8 accumulator banks ---
            nramp = 2 * (8 // 2)  # 2 chunks x 4 m-pair tiles
            ramp_ps = [psum_tile(f"ramp{i}") for i in range(nramp)]

            def ramp_acc(ci, m):
                t = ramp_ps[4 * ci + m // 2]
                return t[:, 256 * (m % 2):256 * (m % 2) + 256]

            # start=True clears the whole bank's has_written bits, so only
            # the bank's first writer (the even m of the pair) may set it;
            # the odd m's k=0 matmul overwrites via the cleared bits.
            for k in range(KT):
                for m in range(KT):
                    lhsT = w_t[:, 0, k, m * P:(m + 1) * P]
                    for ci in range(2):
                        nc.tensor.matmul(
                            ramp_acc(ci, m),
                            lhsT,
                            xu_t[:, k, 256 * ci:256 * ci + 256],
                            start=(k == 0 and m % 2 == 0),
                            stop=(k == KT - 1),
                        )
            # silus in bank-rotation order (c0 m0..m7, then c1 m0..m7)
            for ci in range(2):
                for m in range(KT):
                    nc.scalar.activation(
                        x_stage[:, m, 256 * ci:256 * ci + 256],
                        ramp_acc(ci, m),
                        mybir.ActivationFunctionType.Silu,
                        bias=bias_t[:, m:m + 1],
                    )

            # --- routed steady: m-outer N=512 (+E tail) over cols 512:U ---
            for m in range(KT):
                psums = []
                for pi, (c0, c1) in enumerate(steady_chunks):
                    t = psum_tile(f"pst_{m}_{pi}")
                    psums.append(t[:, :c1 - c0])
                for k in range(KT):
                    lhsT = w_t[:, 0, k, m * P:(m + 1) * P]
                    for pi, (c0, c1) in enumerate(steady_chunks):
                        nc.tensor.matmul(
                            psums[pi],
                            lhsT,
                            xu_t[:, k, c0:c1],
                            start=(k == 0),
                            stop=(k == KT - 1),
                        )
                for pi, (c0, c1) in enumerate(steady_chunks):
                    nc.scalar.activation(
                        x_stage[:, m, c0:c1],
                        psums[pi],
                        mybir.ActivationFunctionType.Silu,
                        bias=bias_t[:, m:m + 1],
                    )
            # one batched store of the whole routed output (4.2KB dst lines);
            # scalar queue is free of input loads by now
            nc.scalar.dma_start(
                outr_d.rearrange("(m p) u -> p m u", p=P), x_stage[:]
            )

            # --- shared experts: j=1,2 over the fixed S-token window.
            # Chunk-outer / j-inner, so the final silu->add->store chain
            # after the very last matmul covers only one 512-col chunk. ---
            for m in range(KT):
                # last m-step ends on a 256-col chunk so the final
                # silu->add->store chain after the last matmul is half-length
                m_chunks = s_chunks if m < KT - 1 else (
                    s_chunks[:-1] + [(s_chunks[-1][0], (s_chunks[-1][0] + s_chunks[-1][1]) // 2),
                                     ((s_chunks[-1][0] + s_chunks[-1][1]) // 2, s_chunks[-1][1])]
                )
                for (c0, c1) in m_chunks:
                    ps1 = psum_tile(f"pss_{m}_{c0}_1")[:, :c1 - c0]
                    ps2 = psum_tile(f"pss_{m}_{c0}_2")[:, :c1 - c0]
                    for k in range(KT):
                        nc.tensor.matmul(
                            ps1, w_t[:, 1, k, m * P:(m + 1) * P],
                            xu_t[:, k, c0:c1],
                            start=(k == 0), stop=(k == KT - 1),
                        )
                        nc.tensor.matmul(
                            ps2, w_t[:, 2, k, m * P:(m + 1) * P],
                            xu_t[:, k, c0:c1],
                            start=(k == 0), stop=(k == KT - 1),
                        )
                    silA = silp.tile([P, 512], BF16, tag="silA",
                                     name=f"sil_{m}_{c0}_1")[:, :c1 - c0]
                    silB = silp.tile([P, 512], BF16, tag="silB",
                                     name=f"sil_{m}_{c0}_2")[:, :c1 - c0]
                    nc.scalar.activation(
                        silA, ps1, mybir.ActivationFunctionType.Silu,
                        bias=bias_t[:, KT + m:KT + m + 1],
                    )
                    nc.scalar.activation(
                        silB, ps2, mybir.ActivationFunctionType.Silu,
                        bias=bias_t[:, 2 * KT + m:2 * KT + m + 1],
                    )
                    outs_t = outp.tile([P, 512], BF16, tag="outs",
                                       name=f"outs_{m}_{c0}")[:, :c1 - c0]
                    nc.vector.tensor_add(outs_t, silA, silB)
                    nc.sync.dma_start(
                        outs_d[m * P:(m + 1) * P, c0:c1], outs_t
                    )

    nc.compile()
    return nc


def _get_program(C):
    if C not in _program_cache:
        _program_cache[C] = _build_program(C)
    return _program_cache[C]


def kernel(x, expert_idx, Wr, br, Ws, bs):
    global last_results, last_nc, last_in_maps

    x = np.asarray(x, dtype=np.float32)
    idx = np.asarray(expert_idx).astype(np.int64)
    Wr = np.asarray(Wr, dtype=np.float32)
    br = np.asarray(br, dtype=np.float32)
    Ws = np.asarray(Ws, dtype=np.float32)
    bs = np.asarray(bs, dtype=np.float32)

    n_tokens = x.shape[0]
    assert x.shape == (N_CORES * S, D), f"unexpected x shape {x.shape}"

    # --- host-side "all-to-all": group tokens by expert ---
    order = np.argsort(idx, kind="stable")
    counts = np.bincount(idx, minlength=N_EXPERTS)
    offsets = np.zeros(N_EXPERTS + 1, dtype=np.int64)
    np.cumsum(counts, out=offsets[1:])

    x_sorted_bf = x[order].astype(ml_dtypes.bfloat16)

    # Core e's xu = [sorted window e*S:(e+1)*S | extras], where extras are
    # the routed (expert-e) tokens falling outside that window. The routed
    # phase computes all U columns with Wr[e]; only the expert-e ones are
    # kept, so the window+extras layout lets one x load serve both phases.
    extras = []
    cols = []  # per core: xu column of each routed token (sorted order)
    for e in range(N_CORES):
        p = np.arange(offsets[e], offsets[e + 1])
        inside = (p >= e * S) & (p < (e + 1) * S)
        ex = p[~inside]
        col = np.where(inside, p - e * S, 0)
        col[~inside] = S + np.arange(len(ex))
        extras.append(ex)
        cols.append(col)
    E = max(32, int(math.ceil(max(len(ex) for ex in extras) / 32)) * 32)
    U = S + E

    Wr_bf = Wr.astype(ml_dtypes.bfloat16)
    Ws_bf = Ws.astype(ml_dtypes.bfloat16)

    in_maps = []
    for e in range(N_CORES):
        xu = np.zeros((D, U), dtype=ml_dtypes.bfloat16)
        xu[:, :S] = x_sorted_bf[e * S:(e + 1) * S].T
        if len(extras[e]):
            xu[:, S:S + len(extras[e])] = x_sorted_bf[extras[e]].T

        W = np.empty((NJ, D, D), dtype=ml_dtypes.bfloat16)
        W[0] = Wr_bf[e]
        W[1] = Ws_bf[0]
        W[2] = Ws_bf[1]

        # b[p, j*KT + m] = bias_j[m*P + p]
        b = np.empty((P, NJ * KT), dtype=np.float32)
        for j, bias in enumerate((br[e], bs[0], bs[1])):
            b[:, j * KT:(j + 1) * KT] = bias.reshape(KT, P).T

        in_maps.append({"xu": xu, "W": W, "b": b})

    nc = _get_program(U)
    res = bass_utils.run_bass_kernel_spmd(nc, in_maps, core_ids=list(range(N_CORES)))
    last_results = res
    last_nc = nc
    last_in_maps = in_maps

    # combine in sorted-token space, then permute back to input order
    out_sorted = np.concatenate(
        [res.results[e]["outs"].T for e in range(N_CORES)], axis=0
    ).astype(np.float32)
    for e in range(N_CORES):
        if counts[e] == 0:
            continue
        out_sorted[offsets[e]:offsets[e + 1]] += res.results[e]["outr"][
            :, cols[e]
        ].T
    out = np.empty_like(out_sorted)
    out[order] = out_sorted
    return out[:n_tokens]
